# revision 16
# baseline (speedup 1.0000x reference)
"""AutoCorrelation block (FFT cross-correlation attention) on 8 Trainium2 cores.

Math (per batch b, faithfully reproducing the reference):
  qh = q @ Wq + bq, kh = k @ Wk + bk         (v projection is dead code)
  per channel c=(h,dh) (512 per batch):
    r = irfft(rfft(qh_c) * conj(rfft(kh_c)))   # circular cross-correlation
    top-8 lags d_k of r, softmax of the 8 values -> w_k
    agg_c[t] = sum_k w_k * qh_c[(t + d_k) % L]
  out = agg^T @ Wo + bo

Implementation: DFT-as-matmul with a stacked real cos/sin basis (the DFT matrix
is shared by all channels, so the whole FFT pipeline is dense PE work), DVE
max/max_index for top-8, and per-partition indirect-DMA gathers from a
time-doubled copy of qh for the mod-L rolls.

Sharding: data-parallel over batch. B == 8 == n_cores, one batch per core,
weights + DFT matrices replicated. No collectives.
"""

import numpy as np

import concourse.bass as bass
import concourse.bacc as bacc
import concourse.mybir as mybir
import concourse.tile as tile
from concourse.bass import IndirectOffsetOnAxis, ts
from concourse.bass_utils import run_bass_kernel_spmd

B, L, D = 8, 2048, 512
TOPK = 8
NF = 1025          # rfft bins for L=2048
FS = 2176          # stacked freq rows: 17 chunks of 128
IM0 = 1152         # sin(f) block starts here (f = 0..1023); rows 1025..1151 zero
N_CORES = 8
KC = 4             # d_in chunks of 128
TM = 16            # time chunks of 128
CN = 4             # channel chunks of 128
FM = 17            # stacked-freq chunks of 128

F32 = mybir.dt.float32
F32R = mybir.dt.float32r
U32 = mybir.dt.uint32
BF16 = mybir.dt.bfloat16
AF = mybir.ActivationFunctionType
AX = mybir.AxisListType


def _build_dft_mats():
    t = np.arange(L)
    f = np.arange(NF)
    ang = (2.0 * np.pi / L) * ((t[:, None] * f[None, :]) % L)
    Cf = np.zeros((L, FS), np.float32)
    Cf[:, :NF] = np.cos(ang)
    Cf[:, IM0 : IM0 + 1024] = np.sin(ang[:, :1024])
    ang2 = (2.0 * np.pi / L) * ((f[:, None] * t[None, :]) % L)
    Mi = np.zeros((FS, L), np.float32)
    Mi[0, :] = 1.0 / L
    Mi[1:1024, :] = (2.0 / L) * np.cos(ang2[1:1024])
    Mi[1024, :] = (1.0 / L) * np.cos(ang2[1024])
    Mi[IM0 : IM0 + 1024, :] = (2.0 / L) * np.sin(ang2[:1024])
    return Cf, Mi


def _kernel_body(tc, dr, out_ap, q2):
    nc = tc.nc

    w_pool = tc.alloc_tile_pool(name="weights", bufs=1)
    htd_pool = tc.alloc_tile_pool(name="htd", bufs=1, side="right")

    # ---- constants ----
    wqk_pool = tc.alloc_tile_pool(name="wqk", bufs=1)
    wq_t = wqk_pool.tile([128, KC * D], F32R, tag="wqt", name="wqt")
    wk_t = wqk_pool.tile([128, KC * D], F32R, tag="wkt", name="wkt")
    wo_t = w_pool.tile([128, KC * D], F32R, tag="wot", name="wot")
    nc.sync.dma_start(wq_t[:, :], dr["Wq"][:, :])
    nc.sync.dma_start(wk_t[:, :], dr["Wk"][:, :])
    nc.sync.dma_start(wo_t[:, :], dr["Wo"][:, :])
    wq = [wq_t[:, ts(i, D)] for i in range(KC)]
    wk = [wk_t[:, ts(i, D)] for i in range(KC)]
    wo = [wo_t[:, ts(i, D)] for i in range(KC)]
    ones = w_pool.tile([1, 128], F32R, tag="ones", name="ones")
    nc.sync.dma_start(ones[:, :], dr["ones"][:, :])
    ident = w_pool.tile([128, 128], BF16, tag="ident", name="ident")
    nc.sync.dma_start(ident[:, :], dr["ident"][:, :])
    brow = {}
    for nm in ("bq", "bk", "bo"):
        brow[nm] = w_pool.tile([1, D], F32R, tag=f"{nm}r", name=f"{nm}r")
        nc.sync.dma_start(brow[nm][:, :], dr[nm][:, :])
    bqcol = w_pool.tile([128, CN], F32, tag="bqc", name="bqc")
    for c in range(CN):
        nc.sync.dma_start(bqcol[:, c : c + 1], dr["bqc"][ts(c, 128), :])

    htd_q = [htd_pool.tile([128, D], F32R, tag=f"hq{m}", name=f"hq{m}") for m in range(TM)]
    htd_k = [htd_pool.tile([128, D], F32R, tag=f"hk{m}", name=f"hk{m}") for m in range(TM)]

    # ---- S1/S2: projections ----
    qt_pool = tc.alloc_tile_pool(name="qt", bufs=1)
    ps1 = tc.alloc_tile_pool(name="ps1", bufs=4, space="PSUM")
    qht_pool = tc.alloc_tile_pool(name="qht", bufs=2)

    qt = [qt_pool.tile([128, L], F32R, tag=f"qt{i}", name=f"qt{i}") for i in range(KC)]
    for i in range(KC):
        nc.sync.dma_start(qt[i][:, :], dr["qT"][ts(i, 128), :])
    # qh_td[t, c] = sum_di qT[di, t] * Wq[di, c]  (+ bq broadcast row)
    for m in range(TM):
        ps = ps1.tile([128, D], F32, tag="p1", name="p1")
        for kc in range(KC):
            nc.tensor.matmul(
                ps[:, :], qt[kc][:, ts(m, 128)], wq[kc],
                start=(kc == 0), stop=False,
            )
        nc.tensor.matmul(
            ps[:, :], ones[:, :], brow["bq"][:, :], start=False, stop=True
        )
        nc.scalar.activation(htd_q[m][:, :], ps[:, :], AF.Copy)
    # qh_t[c, t] (channel-major) -> q2 doubled in DRAM
    for mc in range(CN):
        qht = qht_pool.tile([128, L], BF16, tag="qht", name="qht")
        for n in range(4):
            ps = ps1.tile([128, 512], F32, tag="p1", name="p1")
            for kc in range(KC):
                nc.tensor.matmul(
                    ps[:, :], wq[kc][:, ts(mc, 128)], qt[kc][:, ts(n, 512)],
                    start=(kc == 0), stop=(kc == KC - 1),
                )
            nc.vector.tensor_scalar_add(
                qht[:, ts(n, 512)], ps[:, :], bqcol[:, mc : mc + 1]
            )
        nc.sync.dma_start(q2[ts(mc, 128), 0:L], qht[:, :])
        nc.sync.dma_start(q2[ts(mc, 128), L : 2 * L], qht[:, :])
    # kh_td
    kt = [qt_pool.tile([128, L], F32R, tag=f"qt{i}", name=f"kt{i}") for i in range(KC)]
    for i in range(KC):
        nc.sync.dma_start(kt[i][:, :], dr["kT"][ts(i, 128), :])
    for m in range(TM):
        ps = ps1.tile([128, D], F32, tag="p1", name="p1")
        for kc in range(KC):
            nc.tensor.matmul(
                ps[:, :], kt[kc][:, ts(m, 128)], wk[kc],
                start=(kc == 0), stop=False,
            )
        nc.tensor.matmul(
            ps[:, :], ones[:, :], brow["bk"][:, :], start=False, stop=True
        )
        nc.scalar.activation(htd_k[m][:, :], ps[:, :], AF.Copy)

    qht_pool.release()
    ps1.release()
    qt_pool.release()
    wqk_pool.release()

    # ---- S3+S4 fused: forward DFT with inline freq product ----
    # Qhat[fs, c] = sum_t Cf[t, fs] * qh_td[t, c]; pairs (j, 9+j) are produced
    # back-to-back so Z = Qhat * conj(Khat) is computed inline and the big
    # Qhat/Khat buffers never materialize.
    z_pool = tc.alloc_tile_pool(name="zfreq", bufs=1)
    cf_pool = tc.alloc_tile_pool(name="cf", bufs=2)
    f_pool = tc.alloc_tile_pool(name="fpair", bufs=6)
    ps3 = tc.alloc_tile_pool(name="ps3", bufs=2, space="PSUM")

    Z = [z_pool.tile([128, D], F32R, tag=f"z{j}", name=f"z{j}") for j in range(FM)]

    def dft_chunk(m):
        psq = ps3.tile([128, D], F32, tag="p3q", name="p3q")
        psk = ps3.tile([128, D], F32, tag="p3k", name="p3k")
        cf_t = cf_pool.tile([128, TM * 128], F32R, tag="cf", name="cf")
        nc.sync.dma_start(cf_t[:, :], dr["Cf"][ts(m, 128), :])
        for kc in range(TM):
            nc.tensor.matmul(
                psq[:, :], cf_t[:, ts(kc, 128)], htd_q[kc][:, :],
                start=(kc == 0), stop=(kc == TM - 1),
            )
            nc.tensor.matmul(
                psk[:, :], cf_t[:, ts(kc, 128)], htd_k[kc][:, :],
                start=(kc == 0), stop=(kc == TM - 1),
            )
        qf = f_pool.tile([128, D], F32R, tag="qf", name="qf")
        kf = f_pool.tile([128, D], F32R, tag="kf", name="kf")
        nc.scalar.activation(qf[:, :], psq[:, :], AF.Copy)
        nc.scalar.activation(kf[:, :], psk[:, :], AF.Copy)
        return qf, kf

    for j in range(8):
        re, im = j, 9 + j
        qf_a, kf_a = dft_chunk(re)
        qf_b, kf_b = dft_chunk(im)
        # Zre_j = Qre Kre + Qnim Knim ; Znim_j = Qnim Kre - Qre Knim
        t1 = f_pool.tile([128, D], F32R, tag="zt", name="zt")
        nc.vector.tensor_mul(Z[re][:, :], qf_a[:, :], kf_a[:, :])
        nc.gpsimd.tensor_mul(t1[:, :], qf_b[:, :], kf_b[:, :])
        nc.vector.tensor_add(Z[re][:, :], Z[re][:, :], t1[:, :])
        t2 = f_pool.tile([128, D], F32R, tag="zt", name="zt")
        nc.gpsimd.tensor_mul(t2[:, :], qf_b[:, :], kf_a[:, :])
        nc.vector.tensor_mul(Z[im][:, :], qf_a[:, :], kf_b[:, :])
        nc.vector.tensor_sub(Z[im][:, :], t2[:, :], Z[im][:, :])
    qf_n, kf_n = dft_chunk(8)
    nc.vector.tensor_mul(Z[8][:, :], qf_n[:, :], kf_n[:, :])

    ps3.release()
    f_pool.release()
    cf_pool.release()
    htd_pool.release()

    # ---- S5: inverse DFT  R[c, tau] = sum_fs Z[fs, c] * Mi[fs, tau] ----
    mi_pool = tc.alloc_tile_pool(name="mi", bufs=2)
    r_pool = tc.alloc_tile_pool(name="rcorr", bufs=1, side="right")
    ps5 = tc.alloc_tile_pool(name="ps5", bufs=2, space="PSUM")

    R = [r_pool.tile([128, L], F32, tag=f"r{m}", name=f"r{m}") for m in range(CN)]
    for n in range(4):
        pss = [ps5.tile([128, 512], F32, tag=f"p5{m}", name=f"p5{m}") for m in range(CN)]
        mi_t = mi_pool.tile([128, FM * 512], F32R, tag="mi", name="mi")
        nc.sync.dma_start(mi_t[:, :], dr["Mi"][ts(n, 128), :])
        for kc in range(FM):
            for m in range(CN):
                nc.tensor.matmul(
                    pss[m][:, :], Z[kc][:, ts(m, 128)], mi_t[:, ts(kc, 512)],
                    start=(kc == 0), stop=(kc == FM - 1),
                )
        for m in range(CN):
            nc.scalar.activation(R[m][:, ts(n, 512)], pss[m][:, :], AF.Copy)

    ps5.release()
    mi_pool.release()
    z_pool.release()

    # ---- S6/S7: top-8 + softmax, then roll-gather + weighted sum.
    # The weighted sum runs on the PE: psum += I^T @ (w_k * G_k), keeping
    # the tensor engine warm through the tail instead of burning DVE adds.
    s_pool = tc.alloc_tile_pool(name="small", bufs=1)
    acc_pool = tc.alloc_tile_pool(name="acc", bufs=1, side="right")
    g_pool = tc.alloc_tile_pool(name="g", bufs=3)
    gm_pool = tc.alloc_tile_pool(name="gm", bufs=2)
    psa = tc.alloc_tile_pool(name="psa", bufs=8, space="PSUM")

    acc = [acc_pool.tile([128, L], F32R, tag=f"a{mc}", name=f"a{mc}") for mc in range(CN)]
    wsm = []
    offs = []
    for mc in range(CN):
        vals = s_pool.tile([128, 8], F32, tag=f"v{mc}", name=f"v{mc}")
        nc.vector.max(out=vals[:, :], in_=R[mc][:, :])
        idx = s_pool.tile([128, 8], U32, tag=f"i{mc}", name=f"i{mc}")
        nc.vector.max_index(out=idx[:, :], in_max=vals[:, :], in_values=R[mc][:, :])
        negm = s_pool.tile([128, 1], F32, tag=f"nm{mc}", name=f"nm{mc}")
        nc.vector.tensor_scalar_mul(negm[:, :], vals[:, 0:1], -1.0)
        e = s_pool.tile([128, 8], F32, tag=f"e{mc}", name=f"e{mc}")
        nc.scalar.activation(e[:, :], vals[:, :], AF.Exp, bias=negm[:, :])
        ssum = s_pool.tile([128, 1], F32, tag=f"s{mc}", name=f"s{mc}")
        nc.vector.reduce_sum(out=ssum[:, :], in_=e[:, :], axis=AX.X)
        rs = s_pool.tile([128, 1], F32, tag=f"rs{mc}", name=f"rs{mc}")
        nc.vector.reciprocal(rs[:, :], ssum[:, :])
        wt = s_pool.tile([128, 8], F32, tag=f"w{mc}", name=f"w{mc}")
        nc.vector.tensor_scalar_mul(wt[:, :], e[:, :], rs[:, :])
        iob = s_pool.tile([128, 8], U32, tag=f"io{mc}", name=f"io{mc}")
        nc.gpsimd.iota(
            iob[:, :], pattern=[[0, 8]], base=mc * 128 * 2 * L,
            channel_multiplier=2 * L,
        )
        off = s_pool.tile([128, 8], U32, tag=f"o{mc}", name=f"o{mc}")
        nc.vector.tensor_add(off[:, :], idx[:, :], iob[:, :])
        wsm.append(wt)
        offs.append(off)

    for mc in range(CN):
        pacc = [psa.tile([128, 512], F32, tag="pa", name="pa") for _ in range(4)]
        for k in range(TOPK):
            g = g_pool.tile([128, L], BF16, tag="g", name="g")
            nc.gpsimd.indirect_dma_start(
                out=g[:, :],
                out_offset=None,
                in_=q2[:, :],
                in_offset=IndirectOffsetOnAxis(ap=offs[mc][:, k : k + 1], axis=1),
            )
            gm = gm_pool.tile([128, L], BF16, tag="gm", name="gm")
            nc.scalar.activation(
                gm[:, :], g[:, :], AF.Identity, scale=wsm[mc][:, k : k + 1]
            )
            for nsl in range(4):
                nc.tensor.matmul(
                    pacc[nsl][:, :], ident[:, :], gm[:, ts(nsl, 512)],
                    start=(k == 0), stop=(k == TOPK - 1),
                )
        for nsl in range(4):
            nc.scalar.activation(acc[mc][:, ts(nsl, 512)], pacc[nsl][:, :], AF.Copy)

    psa.release()
    gm_pool.release()
    g_pool.release()
    s_pool.release()

    # ---- S8: output projection  out[t, :] = sum_c acc[c, t] * Wo[c, :] + bo ----
    po_pool = tc.alloc_tile_pool(name="po", bufs=1, space="PSUM")
    ot_pool = tc.alloc_tile_pool(name="ot", bufs=4)
    for half in range(2):
        pss = [po_pool.tile([128, D], F32, tag=f"po{m8}", name=f"po{m8}")
               for m8 in range(8)]
        for kc in range(CN):
            for m8 in range(8):
                nc.tensor.matmul(
                    pss[m8][:, :], acc[kc][:, ts(half * 8 + m8, 128)], wo[kc],
                    start=(kc == 0), stop=False,
                )
        for m8 in range(8):
            nc.tensor.matmul(
                pss[m8][:, :], ones[:, :], brow["bo"][:, :], start=False, stop=True
            )
            ot = ot_pool.tile([128, D], F32, tag="ot", name="ot")
            nc.scalar.activation(ot[:, :], pss[m8][:, :], AF.Copy)
            nc.sync.dma_start(out_ap[ts(half * 8 + m8, 128), :], ot[:, :])

    ot_pool.release()
    po_pool.release()
    acc_pool.release()
    r_pool.release()
    w_pool.release()


def build_module():
    nc = bacc.Bacc(
        "TRN2",
        target_bir_lowering=False,
        debug=False,
        enable_asserts=False,
        num_devices=N_CORES,
    )
    dr = {}

    def din(name, shape, dt=F32R):
        dr[name] = nc.dram_tensor(name, shape, dt, kind="ExternalInput").ap()

    din("qT", [D, L])
    din("kT", [D, L])
    din("Wq", [128, KC * D])   # tiled: [p, kc*D+j] = W[kc*128+p, j]
    din("Wk", [128, KC * D])
    din("Wo", [128, KC * D])
    din("bq", [1, D])
    din("bk", [1, D])
    din("bo", [1, D])
    din("bqc", [D, 1], F32)
    din("ones", [1, 128])
    din("ident", [128, 128], BF16)
    din("Cf", [FM * 128, TM * 128])   # [m*128+p, kc*128+j] = Cf[kc*128+p, m*128+j]
    din("Mi", [4 * 128, FM * 512])    # [n*128+p, kc*512+j] = Mi[kc*128+p, n*512+j]
    out_ap = nc.dram_tensor("out", [L, D], F32, kind="ExternalOutput").ap()
    q2 = nc.dram_tensor("q2", [D, 2 * L], BF16, kind="Internal").ap()

    with tile.TileContext(nc, trace_sim=False) as tc:
        _kernel_body(tc, dr, out_ap, q2)
    nc.compile()
    return nc


_NC_CACHE = {}


def _tile_w(W):
    return np.ascontiguousarray(
        np.asarray(W, np.float32).reshape(KC, 128, D).transpose(1, 0, 2).reshape(128, KC * D)
    )


def make_in_maps(q, k, Wq, bq, Wk, bk, Wo, bo):
    Cf, Mi = _build_dft_mats()
    # pre-tile so each stage does one big contiguous DMA per chunk column
    Cf = np.ascontiguousarray(
        Cf.reshape(TM, 128, FM, 128).transpose(2, 1, 0, 3).reshape(FM * 128, TM * 128)
    )
    Mi = np.ascontiguousarray(
        Mi.reshape(FM, 128, 4, 512).transpose(2, 1, 0, 3).reshape(4 * 128, FM * 512)
    )
    f32 = np.float32
    shared = {
        "Wq": _tile_w(Wq),
        "Wk": _tile_w(Wk),
        "Wo": _tile_w(Wo),
        "bq": np.ascontiguousarray(bq, f32).reshape(1, D),
        "bk": np.ascontiguousarray(bk, f32).reshape(1, D),
        "bo": np.ascontiguousarray(bo, f32).reshape(1, D),
        "bqc": np.ascontiguousarray(bq, f32).reshape(D, 1),
        "ones": np.ones((1, 128), f32),
        "ident": np.eye(128, dtype=np.float32).astype(__import__("ml_dtypes").bfloat16),
        "Cf": Cf,
        "Mi": Mi,
    }
    in_maps = []
    for b in range(B):
        m = dict(shared)
        m["qT"] = np.ascontiguousarray(np.asarray(q[b], f32).T)
        m["kT"] = np.ascontiguousarray(np.asarray(k[b], f32).T)
        in_maps.append(m)
    return in_maps


def kernel(q, k, v, Wq, bq, Wk, bk, Wv, bv, Wo, bo, _want_results=False,
           _trace=False, **_ignored):
    if "nc" not in _NC_CACHE:
        _NC_CACHE["nc"] = build_module()
    nc = _NC_CACHE["nc"]
    in_maps = make_in_maps(q, k, Wq, bq, Wk, bk, Wo, bo)
    res = run_bass_kernel_spmd(
        nc, in_maps, core_ids=list(range(N_CORES)), trace=_trace
    )
    out = np.stack([np.asarray(res.results[b]["out"], np.float32) for b in range(B)])
    if _want_results:
        return out, res
    return out


if __name__ == "__main__":
    # smoke test with random data
    rng = np.random.default_rng(0)
    q = rng.standard_normal((B, L, D), np.float32)
    k = rng.standard_normal((B, L, D), np.float32)
    s = 1.0 / np.sqrt(D)
    Wq = rng.standard_normal((D, D), np.float32) * s
    Wk = rng.standard_normal((D, D), np.float32) * s
    Wo = rng.standard_normal((D, D), np.float32) * s
    z = np.zeros(D, np.float32)
    out = kernel(q, k, None, Wq, z, Wk, z, None, None, Wo, z)
    print("out", out.shape, out.dtype, float(np.abs(out).sum()))


# revision 18
# speedup vs baseline: 1.0955x; 1.0955x over previous
"""AutoCorrelation block (FFT cross-correlation attention) on 8 Trainium2 cores.

Math (per batch b, faithfully reproducing the reference):
  qh = q @ Wq + bq, kh = k @ Wk + bk         (v projection is dead code)
  per channel c=(h,dh) (512 per batch):
    r = irfft(rfft(qh_c) * conj(rfft(kh_c)))   # circular cross-correlation
    top-8 lags d_k of r, softmax of the 8 values -> w_k
    agg_c[t] = sum_k w_k * qh_c[(t + d_k) % L]
  out = agg^T @ Wo + bo

Implementation: DFT-as-matmul with a stacked real cos/sin basis (the DFT matrix
is shared by all channels, so the whole FFT pipeline is dense PE work), DVE
max/max_index for top-8, and per-partition indirect-DMA gathers from a
time-doubled copy of qh for the mod-L rolls.

Sharding: data-parallel over batch. B == 8 == n_cores, one batch per core,
weights + DFT matrices replicated. No collectives.
"""

import numpy as np

import concourse.bass as bass
import concourse.bacc as bacc
import concourse.mybir as mybir
import concourse.tile as tile
from concourse.bass import IndirectOffsetOnAxis, ts
from concourse.bass_utils import run_bass_kernel_spmd

B, L, D = 8, 2048, 512
TOPK = 8
NF = 1025          # rfft bins for L=2048
FS = 2176          # stacked freq rows: 17 chunks of 128
IM0 = 1152         # sin(f) block starts here (f = 0..1023); rows 1025..1151 zero
N_CORES = 8
KC = 4             # d_in chunks of 128
TM = 16            # time chunks of 128
CN = 4             # channel chunks of 128
FM = 17            # stacked-freq chunks of 128

F32 = mybir.dt.float32
F32R = mybir.dt.float32r
U32 = mybir.dt.uint32
BF16 = mybir.dt.bfloat16
AF = mybir.ActivationFunctionType
AX = mybir.AxisListType


def _build_dft_mats():
    t = np.arange(L)
    f = np.arange(NF)
    ang = (2.0 * np.pi / L) * ((t[:, None] * f[None, :]) % L)
    Cf = np.zeros((L, FS), np.float32)
    Cf[:, :NF] = np.cos(ang)
    Cf[:, IM0 : IM0 + 1024] = np.sin(ang[:, :1024])
    ang2 = (2.0 * np.pi / L) * ((f[:, None] * t[None, :]) % L)
    Mi = np.zeros((FS, L), np.float32)
    Mi[0, :] = 1.0 / L
    Mi[1:1024, :] = (2.0 / L) * np.cos(ang2[1:1024])
    Mi[1024, :] = (1.0 / L) * np.cos(ang2[1024])
    Mi[IM0 : IM0 + 1024, :] = (2.0 / L) * np.sin(ang2[:1024])
    return Cf, Mi


def _kernel_body(tc, dr, out_ap, q2):
    nc = tc.nc

    w_pool = tc.alloc_tile_pool(name="weights", bufs=1)
    htd_pool = tc.alloc_tile_pool(name="htd", bufs=1, side="right")

    # ---- S1 inputs first so the PE can start ASAP ----
    qt_pool = tc.alloc_tile_pool(name="qt", bufs=1)
    qt = [qt_pool.tile([128, L], F32R, tag=f"qt{i}", name=f"qt{i}") for i in range(KC)]
    for i in range(KC):
        nc.sync.dma_start(qt[i][:, :], dr["qT"][ts(i, 128), :])

    # ---- constants ----
    wqk_pool = tc.alloc_tile_pool(name="wqk", bufs=1)
    wq_t = wqk_pool.tile([128, KC * D], F32R, tag="wqt", name="wqt")
    wk_t = wqk_pool.tile([128, KC * D], F32R, tag="wkt", name="wkt")
    wo_t = w_pool.tile([128, KC * D], F32R, tag="wot", name="wot")
    nc.sync.dma_start(wq_t[:, :], dr["Wq"][:, :])
    nc.sync.dma_start(wk_t[:, :], dr["Wk"][:, :])
    nc.sync.dma_start(wo_t[:, :], dr["Wo"][:, :])
    wq = [wq_t[:, ts(i, D)] for i in range(KC)]
    wk = [wk_t[:, ts(i, D)] for i in range(KC)]
    wo = [wo_t[:, ts(i, D)] for i in range(KC)]
    ones = w_pool.tile([1, 128], F32R, tag="ones", name="ones")
    nc.sync.dma_start(ones[:, :], dr["ones"][:, :])
    ident = w_pool.tile([128, 128], BF16, tag="ident", name="ident")
    nc.sync.dma_start(ident[:, :], dr["ident"][:, :])
    brow = {}
    for nm in ("bq", "bk", "bo"):
        brow[nm] = w_pool.tile([1, D], F32R, tag=f"{nm}r", name=f"{nm}r")
        nc.sync.dma_start(brow[nm][:, :], dr[nm][:, :])
    bqcol = w_pool.tile([128, CN], F32, tag="bqc", name="bqc")
    for c in range(CN):
        nc.sync.dma_start(bqcol[:, c : c + 1], dr["bqc"][ts(c, 128), :])

    htd_q = [htd_pool.tile([128, D], F32R, tag=f"hq{m}", name=f"hq{m}") for m in range(TM)]
    htd_k = [htd_pool.tile([128, D], F32R, tag=f"hk{m}", name=f"hk{m}") for m in range(TM)]

    # ---- S1/S2: projections ----
    ps1 = tc.alloc_tile_pool(name="ps1", bufs=4, space="PSUM")
    qht_pool = tc.alloc_tile_pool(name="qht", bufs=2)

    # qh_td[t, c] = sum_di qT[di, t] * Wq[di, c]  (+ bq broadcast row)
    for m in range(TM):
        ps = ps1.tile([128, D], F32, tag="p1", name="p1")
        for kc in range(KC):
            nc.tensor.matmul(
                ps[:, :], qt[kc][:, ts(m, 128)], wq[kc],
                start=(kc == 0), stop=False,
            )
        nc.tensor.matmul(
            ps[:, :], ones[:, :], brow["bq"][:, :], start=False, stop=True
        )
        nc.scalar.activation(htd_q[m][:, :], ps[:, :], AF.Copy)
    # qh_t[c, t] (channel-major) -> q2 doubled in DRAM
    for mc in range(CN):
        qht = qht_pool.tile([128, L], BF16, tag="qht", name="qht")
        for n in range(4):
            ps = ps1.tile([128, 512], F32, tag="p1", name="p1")
            for kc in range(KC):
                nc.tensor.matmul(
                    ps[:, :], wq[kc][:, ts(mc, 128)], qt[kc][:, ts(n, 512)],
                    start=(kc == 0), stop=(kc == KC - 1),
                )
            nc.vector.tensor_scalar_add(
                qht[:, ts(n, 512)], ps[:, :], bqcol[:, mc : mc + 1]
            )
        nc.sync.dma_start(q2[ts(mc, 128), 0:L], qht[:, :])
        nc.sync.dma_start(q2[ts(mc, 128), L : 2 * L], qht[:, :])
    # kh_td
    kt = [qt_pool.tile([128, L], F32R, tag=f"qt{i}", name=f"kt{i}") for i in range(KC)]
    for i in range(KC):
        nc.sync.dma_start(kt[i][:, :], dr["kT"][ts(i, 128), :])
    for m in range(TM):
        ps = ps1.tile([128, D], F32, tag="p1", name="p1")
        for kc in range(KC):
            nc.tensor.matmul(
                ps[:, :], kt[kc][:, ts(m, 128)], wk[kc],
                start=(kc == 0), stop=False,
            )
        nc.tensor.matmul(
            ps[:, :], ones[:, :], brow["bk"][:, :], start=False, stop=True
        )
        nc.scalar.activation(htd_k[m][:, :], ps[:, :], AF.Copy)

    qht_pool.release()
    ps1.release()
    wqk_pool.release()
    qt_pool.release()

    # ---- S3+S4 fused: forward DFT with inline freq product ----
    # Qhat[fs, c] = sum_t Cf[t, fs] * qh_td[t, c]; pairs (j, 9+j) are produced
    # back-to-back so Z = Qhat * conj(Khat) is computed inline and the big
    # Qhat/Khat buffers never materialize.
    s_pool0 = tc.alloc_tile_pool(name="small0", bufs=1)
    mi0_pool = tc.alloc_tile_pool(name="mi0", bufs=1)
    mi0_t = mi0_pool.tile([128, FM * 512], F32R, tag="mi0", name="mi0")
    nc.sync.dma_start(mi0_t[:, :], dr["Mi"][ts(0, 128), :])

    z_pool = tc.alloc_tile_pool(name="zfreq", bufs=1)
    cf_pool = tc.alloc_tile_pool(name="cf", bufs=2)
    f_pool = tc.alloc_tile_pool(name="fpair", bufs=6)
    ps3 = tc.alloc_tile_pool(name="ps3", bufs=2, space="PSUM")

    Z = [z_pool.tile([128, D], F32R, tag=f"z{j}", name=f"z{j}") for j in range(FM)]

    def dft_chunk(m):
        psq = ps3.tile([128, D], F32, tag="p3q", name="p3q")
        psk = ps3.tile([128, D], F32, tag="p3k", name="p3k")
        cf_t = cf_pool.tile([128, TM * 128], F32R, tag="cf", name="cf")
        nc.sync.dma_start(cf_t[:, :], dr["Cf"][ts(m, 128), :])
        for kc in range(TM):
            nc.tensor.matmul(
                psq[:, :], cf_t[:, ts(kc, 128)], htd_q[kc][:, :],
                start=(kc == 0), stop=(kc == TM - 1),
            )
            nc.tensor.matmul(
                psk[:, :], cf_t[:, ts(kc, 128)], htd_k[kc][:, :],
                start=(kc == 0), stop=(kc == TM - 1),
            )
        qf = f_pool.tile([128, D], F32R, tag="qf", name="qf")
        kf = f_pool.tile([128, D], F32R, tag="kf", name="kf")
        nc.scalar.activation(qf[:, :], psq[:, :], AF.Copy)
        nc.scalar.activation(kf[:, :], psk[:, :], AF.Copy)
        return qf, kf

    for j in range(8):
        re, im = j, 9 + j
        qf_a, kf_a = dft_chunk(re)
        qf_b, kf_b = dft_chunk(im)
        # Zre_j = Qre Kre + Qnim Knim ; Znim_j = Qnim Kre - Qre Knim
        t1 = f_pool.tile([128, D], F32R, tag="zt", name="zt")
        nc.vector.tensor_mul(Z[re][:, :], qf_a[:, :], kf_a[:, :])
        nc.gpsimd.tensor_mul(t1[:, :], qf_b[:, :], kf_b[:, :])
        nc.vector.tensor_add(Z[re][:, :], Z[re][:, :], t1[:, :])
        t2 = f_pool.tile([128, D], F32R, tag="zt", name="zt")
        nc.gpsimd.tensor_mul(t2[:, :], qf_b[:, :], kf_a[:, :])
        nc.vector.tensor_mul(Z[im][:, :], qf_a[:, :], kf_b[:, :])
        nc.vector.tensor_sub(Z[im][:, :], t2[:, :], Z[im][:, :])
    qf_n, kf_n = dft_chunk(8)
    nc.vector.tensor_mul(Z[8][:, :], qf_n[:, :], kf_n[:, :])

    ps3.release()
    f_pool.release()
    cf_pool.release()
    htd_pool.release()

    # ---- S5: inverse DFT  R[c, tau] = sum_fs Z[fs, c] * Mi[fs, tau] ----
    mi_pool = tc.alloc_tile_pool(name="mi", bufs=2)
    r_pool = tc.alloc_tile_pool(name="rcorr", bufs=1, side="right")
    ps5 = tc.alloc_tile_pool(name="ps5", bufs=2, space="PSUM")

    R = [r_pool.tile([128, L], F32, tag=f"r{m}", name=f"r{m}") for m in range(CN)]
    cand = [s_pool0.tile([128, 32], F32, tag=f"c{m}", name=f"c{m}") for m in range(CN)]
    for n in range(4):
        pss = [ps5.tile([128, 512], F32, tag=f"p5{m}", name=f"p5{m}") for m in range(CN)]
        if n == 0:
            mi_t = mi0_t
        else:
            mi_t = mi_pool.tile([128, FM * 512], F32R, tag="mi", name="mi")
            nc.sync.dma_start(mi_t[:, :], dr["Mi"][ts(n, 128), :])
        for kc in range(FM):
            for m in range(CN):
                nc.tensor.matmul(
                    pss[m][:, :], Z[kc][:, ts(m, 128)], mi_t[:, ts(kc, 512)],
                    start=(kc == 0), stop=(kc == FM - 1),
                )
        for m in range(CN):
            nc.scalar.activation(R[m][:, ts(n, 512)], pss[m][:, :], AF.Copy)
            # per-slice top-8 candidates, hidden under the next n's matmuls
            nc.vector.max(out=cand[m][:, ts(n, 8)], in_=R[m][:, ts(n, 512)])

    ps5.release()
    mi_pool.release()
    z_pool.release()
    mi0_pool.release()

    # ---- S6/S7: top-8 + softmax, then roll-gather + weighted sum.
    # The weighted sum runs on the PE: psum += I^T @ (w_k * G_k), keeping
    # the tensor engine warm through the tail instead of burning DVE adds.
    s_pool = tc.alloc_tile_pool(name="small", bufs=1)
    acc_pool = tc.alloc_tile_pool(name="acc", bufs=1, side="right")
    g_pool = tc.alloc_tile_pool(name="g", bufs=3)
    gm_pool = tc.alloc_tile_pool(name="gm", bufs=2)
    psa = tc.alloc_tile_pool(name="psa", bufs=8, space="PSUM")

    acc = [acc_pool.tile([128, L], F32R, tag=f"a{mc}", name=f"a{mc}") for mc in range(CN)]
    wsm = []
    offs = []
    for mc in range(CN):
        vals = s_pool.tile([128, 8], F32, tag=f"v{mc}", name=f"v{mc}")
        nc.vector.max(out=vals[:, :], in_=cand[mc][:, :])
        idx = s_pool.tile([128, 8], U32, tag=f"i{mc}", name=f"i{mc}")
        nc.vector.max_index(out=idx[:, :], in_max=vals[:, :], in_values=R[mc][:, :])
        negm = s_pool.tile([128, 1], F32, tag=f"nm{mc}", name=f"nm{mc}")
        nc.vector.tensor_scalar_mul(negm[:, :], vals[:, 0:1], -1.0)
        e = s_pool.tile([128, 8], F32, tag=f"e{mc}", name=f"e{mc}")
        nc.scalar.activation(e[:, :], vals[:, :], AF.Exp, bias=negm[:, :])
        ssum = s_pool.tile([128, 1], F32, tag=f"s{mc}", name=f"s{mc}")
        nc.vector.reduce_sum(out=ssum[:, :], in_=e[:, :], axis=AX.X)
        rs = s_pool.tile([128, 1], F32, tag=f"rs{mc}", name=f"rs{mc}")
        nc.vector.reciprocal(rs[:, :], ssum[:, :])
        wt = s_pool.tile([128, 8], F32, tag=f"w{mc}", name=f"w{mc}")
        nc.vector.tensor_scalar_mul(wt[:, :], e[:, :], rs[:, :])
        iob = s_pool.tile([128, 8], U32, tag=f"io{mc}", name=f"io{mc}")
        nc.gpsimd.iota(
            iob[:, :], pattern=[[0, 8]], base=mc * 128 * 2 * L,
            channel_multiplier=2 * L,
        )
        off = s_pool.tile([128, 8], U32, tag=f"o{mc}", name=f"o{mc}")
        nc.vector.tensor_add(off[:, :], idx[:, :], iob[:, :])
        wsm.append(wt)
        offs.append(off)

    for mc in range(CN):
        pacc = [psa.tile([128, 512], F32, tag="pa", name="pa") for _ in range(4)]
        for k in range(TOPK):
            g = g_pool.tile([128, L], BF16, tag="g", name="g")
            nc.gpsimd.indirect_dma_start(
                out=g[:, :],
                out_offset=None,
                in_=q2[:, :],
                in_offset=IndirectOffsetOnAxis(ap=offs[mc][:, k : k + 1], axis=1),
            )
            gm = gm_pool.tile([128, L], BF16, tag="gm", name="gm")
            nc.scalar.activation(
                gm[:, :], g[:, :], AF.Identity, scale=wsm[mc][:, k : k + 1]
            )
            for nsl in range(4):
                nc.tensor.matmul(
                    pacc[nsl][:, :], ident[:, :], gm[:, ts(nsl, 512)],
                    start=(k == 0), stop=(k == TOPK - 1),
                )
        for nsl in range(4):
            nc.scalar.activation(acc[mc][:, ts(nsl, 512)], pacc[nsl][:, :], AF.Copy)

    psa.release()
    gm_pool.release()
    g_pool.release()
    s_pool.release()
    s_pool0.release()

    # ---- S8: output projection  out[t, :] = sum_c acc[c, t] * Wo[c, :] + bo ----
    po_pool = tc.alloc_tile_pool(name="po", bufs=1, space="PSUM")
    ot_pool = tc.alloc_tile_pool(name="ot", bufs=4)
    for half in range(2):
        pss = [po_pool.tile([128, D], F32, tag=f"po{m8}", name=f"po{m8}")
               for m8 in range(8)]
        for kc in range(CN):
            for m8 in range(8):
                nc.tensor.matmul(
                    pss[m8][:, :], acc[kc][:, ts(half * 8 + m8, 128)], wo[kc],
                    start=(kc == 0), stop=False,
                )
        for m8 in range(8):
            nc.tensor.matmul(
                pss[m8][:, :], ones[:, :], brow["bo"][:, :], start=False, stop=True
            )
            ot = ot_pool.tile([128, D], F32, tag="ot", name="ot")
            nc.scalar.activation(ot[:, :], pss[m8][:, :], AF.Copy)
            nc.sync.dma_start(out_ap[ts(half * 8 + m8, 128), :], ot[:, :])

    ot_pool.release()
    po_pool.release()
    acc_pool.release()
    r_pool.release()
    w_pool.release()


def build_module():
    nc = bacc.Bacc(
        "TRN2",
        target_bir_lowering=False,
        debug=False,
        enable_asserts=False,
        num_devices=N_CORES,
    )
    dr = {}

    def din(name, shape, dt=F32R):
        dr[name] = nc.dram_tensor(name, shape, dt, kind="ExternalInput").ap()

    din("qT", [D, L])
    din("kT", [D, L])
    din("Wq", [128, KC * D])   # tiled: [p, kc*D+j] = W[kc*128+p, j]
    din("Wk", [128, KC * D])
    din("Wo", [128, KC * D])
    din("bq", [1, D])
    din("bk", [1, D])
    din("bo", [1, D])
    din("bqc", [D, 1], F32)
    din("ones", [1, 128])
    din("ident", [128, 128], BF16)
    din("Cf", [FM * 128, TM * 128])   # [m*128+p, kc*128+j] = Cf[kc*128+p, m*128+j]
    din("Mi", [4 * 128, FM * 512])    # [n*128+p, kc*512+j] = Mi[kc*128+p, n*512+j]
    out_ap = nc.dram_tensor("out", [L, D], F32, kind="ExternalOutput").ap()
    q2 = nc.dram_tensor("q2", [D, 2 * L], BF16, kind="Internal").ap()

    with tile.TileContext(nc, trace_sim=False) as tc:
        _kernel_body(tc, dr, out_ap, q2)
    nc.compile()
    return nc


_NC_CACHE = {}


def _tile_w(W):
    return np.ascontiguousarray(
        np.asarray(W, np.float32).reshape(KC, 128, D).transpose(1, 0, 2).reshape(128, KC * D)
    )


def make_in_maps(q, k, Wq, bq, Wk, bk, Wo, bo):
    Cf, Mi = _build_dft_mats()
    # pre-tile so each stage does one big contiguous DMA per chunk column
    Cf = np.ascontiguousarray(
        Cf.reshape(TM, 128, FM, 128).transpose(2, 1, 0, 3).reshape(FM * 128, TM * 128)
    )
    Mi = np.ascontiguousarray(
        Mi.reshape(FM, 128, 4, 512).transpose(2, 1, 0, 3).reshape(4 * 128, FM * 512)
    )
    f32 = np.float32
    shared = {
        "Wq": _tile_w(Wq),
        "Wk": _tile_w(Wk),
        "Wo": _tile_w(Wo),
        "bq": np.ascontiguousarray(bq, f32).reshape(1, D),
        "bk": np.ascontiguousarray(bk, f32).reshape(1, D),
        "bo": np.ascontiguousarray(bo, f32).reshape(1, D),
        "bqc": np.ascontiguousarray(bq, f32).reshape(D, 1),
        "ones": np.ones((1, 128), f32),
        "ident": np.eye(128, dtype=np.float32).astype(__import__("ml_dtypes").bfloat16),
        "Cf": Cf,
        "Mi": Mi,
    }
    in_maps = []
    for b in range(B):
        m = dict(shared)
        m["qT"] = np.ascontiguousarray(np.asarray(q[b], f32).T)
        m["kT"] = np.ascontiguousarray(np.asarray(k[b], f32).T)
        in_maps.append(m)
    return in_maps


def kernel(q, k, v, Wq, bq, Wk, bk, Wv, bv, Wo, bo, _want_results=False,
           _trace=False, **_ignored):
    if "nc" not in _NC_CACHE:
        _NC_CACHE["nc"] = build_module()
    nc = _NC_CACHE["nc"]
    in_maps = make_in_maps(q, k, Wq, bq, Wk, bk, Wo, bo)
    res = run_bass_kernel_spmd(
        nc, in_maps, core_ids=list(range(N_CORES)), trace=_trace
    )
    out = np.stack([np.asarray(res.results[b]["out"], np.float32) for b in range(B)])
    if _want_results:
        return out, res
    return out


if __name__ == "__main__":
    # smoke test with random data
    rng = np.random.default_rng(0)
    q = rng.standard_normal((B, L, D), np.float32)
    k = rng.standard_normal((B, L, D), np.float32)
    s = 1.0 / np.sqrt(D)
    Wq = rng.standard_normal((D, D), np.float32) * s
    Wk = rng.standard_normal((D, D), np.float32) * s
    Wo = rng.standard_normal((D, D), np.float32) * s
    z = np.zeros(D, np.float32)
    out = kernel(q, k, None, Wq, z, Wk, z, None, None, Wo, z)
    print("out", out.shape, out.dtype, float(np.abs(out).sum()))


# revision 19
# speedup vs baseline: 1.1471x; 1.0471x over previous
"""AutoCorrelation block (FFT cross-correlation attention) on 8 Trainium2 cores.

Math (per batch b, faithfully reproducing the reference):
  qh = q @ Wq + bq, kh = k @ Wk + bk         (v projection is dead code)
  per channel c=(h,dh) (512 per batch):
    r = irfft(rfft(qh_c) * conj(rfft(kh_c)))   # circular cross-correlation
    top-8 lags d_k of r, softmax of the 8 values -> w_k
    agg_c[t] = sum_k w_k * qh_c[(t + d_k) % L]
  out = agg^T @ Wo + bo

Implementation: DFT-as-matmul with a stacked real cos/sin basis (the DFT matrix
is shared by all channels, so the whole FFT pipeline is dense PE work), DVE
max/max_index for top-8, and per-partition indirect-DMA gathers from a
time-doubled copy of qh for the mod-L rolls.

Sharding: data-parallel over batch. B == 8 == n_cores, one batch per core,
weights + DFT matrices replicated. No collectives.
"""

import numpy as np

import concourse.bass as bass
import concourse.bacc as bacc
import concourse.mybir as mybir
import concourse.tile as tile
from concourse.bass import IndirectOffsetOnAxis, ts
from concourse.bass_utils import run_bass_kernel_spmd

B, L, D = 8, 2048, 512
TOPK = 8
NF = 1025          # rfft bins for L=2048
FS = 2176          # stacked freq rows: 17 chunks of 128
IM0 = 1152         # sin(f) block starts here (f = 0..1023); rows 1025..1151 zero
N_CORES = 8
KC = 4             # d_in chunks of 128
TM = 16            # time chunks of 128
CN = 4             # channel chunks of 128
FM = 17            # stacked-freq chunks of 128

F32 = mybir.dt.float32
F32R = mybir.dt.float32r
U32 = mybir.dt.uint32
BF16 = mybir.dt.bfloat16
AF = mybir.ActivationFunctionType
AX = mybir.AxisListType


def _build_dft_mats():
    t = np.arange(L)
    f = np.arange(NF)
    ang = (2.0 * np.pi / L) * ((t[:, None] * f[None, :]) % L)
    Cf = np.zeros((L, FS), np.float32)
    Cf[:, :NF] = np.cos(ang)
    Cf[:, IM0 : IM0 + 1024] = np.sin(ang[:, :1024])
    ang2 = (2.0 * np.pi / L) * ((f[:, None] * t[None, :]) % L)
    Mi = np.zeros((FS, L), np.float32)
    Mi[0, :] = 1.0 / L
    Mi[1:1024, :] = (2.0 / L) * np.cos(ang2[1:1024])
    Mi[1024, :] = (1.0 / L) * np.cos(ang2[1024])
    Mi[IM0 : IM0 + 1024, :] = (2.0 / L) * np.sin(ang2[:1024])
    return Cf, Mi


def _kernel_body(tc, dr, out_ap, q2):
    nc = tc.nc

    w_pool = tc.alloc_tile_pool(name="weights", bufs=1)
    htd_pool = tc.alloc_tile_pool(name="htd", bufs=1, side="right")

    # ---- S1 inputs first so the PE can start ASAP ----
    qt_pool = tc.alloc_tile_pool(name="qt", bufs=1)
    qt = [qt_pool.tile([128, L], F32R, tag=f"qt{i}", name=f"qt{i}") for i in range(KC)]
    for i in range(KC):
        nc.sync.dma_start(qt[i][:, :], dr["qT"][ts(i, 128), :])

    # ---- constants ----
    wqk_pool = tc.alloc_tile_pool(name="wqk", bufs=1)
    wq_t = wqk_pool.tile([128, KC * D], F32R, tag="wqt", name="wqt")
    wk_t = wqk_pool.tile([128, KC * D], F32R, tag="wkt", name="wkt")
    wo_t = w_pool.tile([128, KC * D], F32R, tag="wot", name="wot")
    nc.sync.dma_start(wq_t[:, :], dr["Wq"][:, :])
    nc.sync.dma_start(wk_t[:, :], dr["Wk"][:, :])
    nc.sync.dma_start(wo_t[:, :], dr["Wo"][:, :])
    wq = [wq_t[:, ts(i, D)] for i in range(KC)]
    wk = [wk_t[:, ts(i, D)] for i in range(KC)]
    wo = [wo_t[:, ts(i, D)] for i in range(KC)]
    ones = w_pool.tile([1, 128], F32R, tag="ones", name="ones")
    nc.sync.dma_start(ones[:, :], dr["ones"][:, :])
    ident = w_pool.tile([128, 128], BF16, tag="ident", name="ident")
    nc.sync.dma_start(ident[:, :], dr["ident"][:, :])
    brow = {}
    for nm in ("bq", "bk", "bo"):
        brow[nm] = w_pool.tile([1, D], F32R, tag=f"{nm}r", name=f"{nm}r")
        nc.sync.dma_start(brow[nm][:, :], dr[nm][:, :])
    bqcol = w_pool.tile([128, CN], F32, tag="bqc", name="bqc")
    for c in range(CN):
        nc.sync.dma_start(bqcol[:, c : c + 1], dr["bqc"][ts(c, 128), :])

    htd_q = [htd_pool.tile([128, D], F32R, tag=f"hq{m}", name=f"hq{m}") for m in range(TM)]
    htd_k = [htd_pool.tile([128, D], F32R, tag=f"hk{m}", name=f"hk{m}") for m in range(TM)]

    # ---- S1/S2: projections ----
    ps1 = tc.alloc_tile_pool(name="ps1", bufs=4, space="PSUM")
    qht_pool = tc.alloc_tile_pool(name="qht", bufs=2)

    # qh_td[t, c] = sum_di qT[di, t] * Wq[di, c]  (+ bq broadcast row)
    for m in range(TM):
        ps = ps1.tile([128, D], F32, tag="p1", name="p1")
        for kc in range(KC):
            nc.tensor.matmul(
                ps[:, :], qt[kc][:, ts(m, 128)], wq[kc],
                start=(kc == 0), stop=False,
            )
        nc.tensor.matmul(
            ps[:, :], ones[:, :], brow["bq"][:, :], start=False, stop=True
        )
        nc.scalar.activation(htd_q[m][:, :], ps[:, :], AF.Copy)
    # qh_t[c, t] (channel-major) -> q2 doubled in DRAM
    for mc in range(CN):
        qht = qht_pool.tile([128, L], BF16, tag="qht", name="qht")
        for n in range(4):
            ps = ps1.tile([128, 512], F32, tag="p1", name="p1")
            for kc in range(KC):
                nc.tensor.matmul(
                    ps[:, :], wq[kc][:, ts(mc, 128)], qt[kc][:, ts(n, 512)],
                    start=(kc == 0), stop=(kc == KC - 1),
                )
            nc.vector.tensor_scalar_add(
                qht[:, ts(n, 512)], ps[:, :], bqcol[:, mc : mc + 1]
            )
        nc.sync.dma_start(q2[ts(mc, 128), 0:L], qht[:, :])
        nc.sync.dma_start(q2[ts(mc, 128), L : 2 * L], qht[:, :])
    # kh_td
    kt = [qt_pool.tile([128, L], F32R, tag=f"qt{i}", name=f"kt{i}") for i in range(KC)]
    for i in range(KC):
        nc.sync.dma_start(kt[i][:, :], dr["kT"][ts(i, 128), :])
    for m in range(TM):
        ps = ps1.tile([128, D], F32, tag="p1", name="p1")
        for kc in range(KC):
            nc.tensor.matmul(
                ps[:, :], kt[kc][:, ts(m, 128)], wk[kc],
                start=(kc == 0), stop=False,
            )
        nc.tensor.matmul(
            ps[:, :], ones[:, :], brow["bk"][:, :], start=False, stop=True
        )
        nc.scalar.activation(htd_k[m][:, :], ps[:, :], AF.Copy)

    qht_pool.release()
    ps1.release()
    wqk_pool.release()
    qt_pool.release()

    # ---- S3+S4 fused: forward DFT with inline freq product ----
    # Qhat[fs, c] = sum_t Cf[t, fs] * qh_td[t, c]; pairs (j, 9+j) are produced
    # back-to-back so Z = Qhat * conj(Khat) is computed inline and the big
    # Qhat/Khat buffers never materialize.
    s_pool0 = tc.alloc_tile_pool(name="small0", bufs=1)
    mi0_pool = tc.alloc_tile_pool(name="mi0", bufs=1)
    mi0_t = mi0_pool.tile([128, FM * 512], F32R, tag="mi0", name="mi0")
    nc.sync.dma_start(mi0_t[:, :], dr["Mi"][ts(0, 128), :])

    z_pool = tc.alloc_tile_pool(name="zfreq", bufs=1)
    cf_pool = tc.alloc_tile_pool(name="cf", bufs=2)
    f_pool = tc.alloc_tile_pool(name="fpair", bufs=6)
    ps3 = tc.alloc_tile_pool(name="ps3", bufs=2, space="PSUM")

    Z = [z_pool.tile([128, D], F32R, tag=f"z{j}", name=f"z{j}") for j in range(FM)]

    def dft_chunk(m):
        psq = ps3.tile([128, D], F32, tag="p3q", name="p3q")
        psk = ps3.tile([128, D], F32, tag="p3k", name="p3k")
        cf_t = cf_pool.tile([128, TM * 128], F32R, tag="cf", name="cf")
        nc.sync.dma_start(cf_t[:, :], dr["Cf"][ts(m, 128), :])
        for kc in range(TM):
            nc.tensor.matmul(
                psq[:, :], cf_t[:, ts(kc, 128)], htd_q[kc][:, :],
                start=(kc == 0), stop=(kc == TM - 1),
            )
            nc.tensor.matmul(
                psk[:, :], cf_t[:, ts(kc, 128)], htd_k[kc][:, :],
                start=(kc == 0), stop=(kc == TM - 1),
            )
        qf = f_pool.tile([128, D], F32R, tag="qf", name="qf")
        kf = f_pool.tile([128, D], F32R, tag="kf", name="kf")
        nc.scalar.activation(qf[:, :], psq[:, :], AF.Copy)
        nc.scalar.activation(kf[:, :], psk[:, :], AF.Copy)
        return qf, kf

    for j in range(8):
        re, im = j, 9 + j
        qf_a, kf_a = dft_chunk(re)
        qf_b, kf_b = dft_chunk(im)
        # Zre_j = Qre Kre + Qnim Knim ; Znim_j = Qnim Kre - Qre Knim
        t1 = f_pool.tile([128, D], F32R, tag="zt", name="zt")
        nc.vector.tensor_mul(Z[re][:, :], qf_a[:, :], kf_a[:, :])
        nc.gpsimd.tensor_mul(t1[:, :], qf_b[:, :], kf_b[:, :])
        nc.vector.tensor_add(Z[re][:, :], Z[re][:, :], t1[:, :])
        t2 = f_pool.tile([128, D], F32R, tag="zt", name="zt")
        nc.gpsimd.tensor_mul(t2[:, :], qf_b[:, :], kf_a[:, :])
        nc.vector.tensor_mul(Z[im][:, :], qf_a[:, :], kf_b[:, :])
        nc.vector.tensor_sub(Z[im][:, :], t2[:, :], Z[im][:, :])
    qf_n, kf_n = dft_chunk(8)
    nc.vector.tensor_mul(Z[8][:, :], qf_n[:, :], kf_n[:, :])

    ps3.release()
    f_pool.release()
    cf_pool.release()
    htd_pool.release()

    # ---- S5: inverse DFT  R[c, tau] = sum_fs Z[fs, c] * Mi[fs, tau] ----
    mi_pool = tc.alloc_tile_pool(name="mi", bufs=2)
    r_pool = tc.alloc_tile_pool(name="rcorr", bufs=1, side="right")
    ps5 = tc.alloc_tile_pool(name="ps5", bufs=2, space="PSUM")

    R = [r_pool.tile([128, L], F32, tag=f"r{m}", name=f"r{m}") for m in range(CN)]
    cand = [s_pool0.tile([128, 32], F32, tag=f"c{m}", name=f"c{m}") for m in range(CN)]
    for n in range(4):
        pss = [ps5.tile([128, 512], F32, tag=f"p5{m}", name=f"p5{m}") for m in range(CN)]
        if n == 0:
            mi_t = mi0_t
        else:
            mi_t = mi_pool.tile([128, FM * 512], F32R, tag="mi", name="mi")
            nc.sync.dma_start(mi_t[:, :], dr["Mi"][ts(n, 128), :])
        for kc in range(FM):
            for m in range(CN):
                nc.tensor.matmul(
                    pss[m][:, :], Z[kc][:, ts(m, 128)], mi_t[:, ts(kc, 512)],
                    start=(kc == 0), stop=(kc == FM - 1),
                )
        for m in range(CN):
            nc.scalar.activation(R[m][:, ts(n, 512)], pss[m][:, :], AF.Copy)
            # per-slice top-8 candidates, hidden under the next n's matmuls
            nc.vector.max(out=cand[m][:, ts(n, 8)], in_=R[m][:, ts(n, 512)])

    ps5.release()
    mi_pool.release()
    z_pool.release()
    mi0_pool.release()

    # ---- S6/S7: top-8 + softmax, then roll-gather + weighted sum.
    # The weighted sum runs on the PE: psum += I^T @ (w_k * G_k), keeping
    # the tensor engine warm through the tail instead of burning DVE adds.
    s_pool = tc.alloc_tile_pool(name="small", bufs=1)
    acc_pool = tc.alloc_tile_pool(name="acc", bufs=1, side="right")
    g_pool = tc.alloc_tile_pool(name="g", bufs=3)
    gm_pool = tc.alloc_tile_pool(name="gm", bufs=2)
    psa = tc.alloc_tile_pool(name="psa", bufs=8, space="PSUM")

    acc = [acc_pool.tile([128, L], F32R, tag=f"a{mc}", name=f"a{mc}") for mc in range(CN)]
    wsm = []
    offs = []
    for mc in range(CN):
        vals = s_pool.tile([128, 8], F32, tag=f"v{mc}", name=f"v{mc}")
        nc.vector.max(out=vals[:, :], in_=cand[mc][:, :])
        idx = s_pool.tile([128, 8], U32, tag=f"i{mc}", name=f"i{mc}")
        nc.vector.max_index(out=idx[:, :], in_max=vals[:, :], in_values=R[mc][:, :])
        negm = s_pool.tile([128, 1], F32, tag=f"nm{mc}", name=f"nm{mc}")
        nc.vector.tensor_scalar_mul(negm[:, :], vals[:, 0:1], -1.0)
        e = s_pool.tile([128, 8], F32, tag=f"e{mc}", name=f"e{mc}")
        nc.scalar.activation(e[:, :], vals[:, :], AF.Exp, bias=negm[:, :])
        ssum = s_pool.tile([128, 1], F32, tag=f"s{mc}", name=f"s{mc}")
        nc.vector.reduce_sum(out=ssum[:, :], in_=e[:, :], axis=AX.X)
        rs = s_pool.tile([128, 1], F32, tag=f"rs{mc}", name=f"rs{mc}")
        nc.vector.reciprocal(rs[:, :], ssum[:, :])
        wt = s_pool.tile([128, 8], F32, tag=f"w{mc}", name=f"w{mc}")
        nc.vector.tensor_scalar_mul(wt[:, :], e[:, :], rs[:, :])
        iob = s_pool.tile([128, 8], U32, tag=f"io{mc}", name=f"io{mc}")
        nc.gpsimd.iota(
            iob[:, :], pattern=[[0, 8]], base=mc * 128 * 2 * L,
            channel_multiplier=2 * L,
        )
        off = s_pool.tile([128, 8], U32, tag=f"o{mc}", name=f"o{mc}")
        nc.vector.tensor_add(off[:, :], idx[:, :], iob[:, :])
        wsm.append(wt)
        offs.append(off)

    for mc in range(CN):
        pacc = [psa.tile([128, 512], F32, tag="pa", name="pa") for _ in range(4)]
        for k in range(TOPK):
            g = g_pool.tile([128, L], BF16, tag="g", name="g")
            gi = nc.gpsimd.indirect_dma_start(
                out=g[:, :],
                out_offset=None,
                in_=q2[:, :],
                in_offset=IndirectOffsetOnAxis(ap=offs[mc][:, k : k + 1], axis=1),
            )
            if k % 2:
                gi.ins.queue = "qPoolDynamic1"
            gm = gm_pool.tile([128, L], BF16, tag="gm", name="gm")
            nc.scalar.activation(
                gm[:, :], g[:, :], AF.Identity, scale=wsm[mc][:, k : k + 1]
            )
            for nsl in range(4):
                nc.tensor.matmul(
                    pacc[nsl][:, :], ident[:, :], gm[:, ts(nsl, 512)],
                    start=(k == 0), stop=(k == TOPK - 1),
                )
        for nsl in range(4):
            nc.scalar.activation(acc[mc][:, ts(nsl, 512)], pacc[nsl][:, :], AF.Copy)

    psa.release()
    gm_pool.release()
    g_pool.release()
    s_pool.release()
    s_pool0.release()

    # ---- S8: output projection  out[t, :] = sum_c acc[c, t] * Wo[c, :] + bo ----
    po_pool = tc.alloc_tile_pool(name="po", bufs=1, space="PSUM")
    ot_pool = tc.alloc_tile_pool(name="ot", bufs=4)
    for half in range(2):
        pss = [po_pool.tile([128, D], F32, tag=f"po{m8}", name=f"po{m8}")
               for m8 in range(8)]
        for kc in range(CN):
            for m8 in range(8):
                nc.tensor.matmul(
                    pss[m8][:, :], acc[kc][:, ts(half * 8 + m8, 128)], wo[kc],
                    start=(kc == 0), stop=False,
                )
        for m8 in range(8):
            nc.tensor.matmul(
                pss[m8][:, :], ones[:, :], brow["bo"][:, :], start=False, stop=True
            )
            ot = ot_pool.tile([128, D], F32, tag="ot", name="ot")
            nc.scalar.activation(ot[:, :], pss[m8][:, :], AF.Copy)
            nc.sync.dma_start(out_ap[ts(half * 8 + m8, 128), :], ot[:, :])

    ot_pool.release()
    po_pool.release()
    acc_pool.release()
    r_pool.release()
    w_pool.release()


def build_module():
    nc = bacc.Bacc(
        "TRN2",
        target_bir_lowering=False,
        debug=False,
        enable_asserts=False,
        num_devices=N_CORES,
        num_swdge_queues=2,
    )
    dr = {}

    def din(name, shape, dt=F32R):
        dr[name] = nc.dram_tensor(name, shape, dt, kind="ExternalInput").ap()

    din("qT", [D, L])
    din("kT", [D, L])
    din("Wq", [128, KC * D])   # tiled: [p, kc*D+j] = W[kc*128+p, j]
    din("Wk", [128, KC * D])
    din("Wo", [128, KC * D])
    din("bq", [1, D])
    din("bk", [1, D])
    din("bo", [1, D])
    din("bqc", [D, 1], F32)
    din("ones", [1, 128])
    din("ident", [128, 128], BF16)
    din("Cf", [FM * 128, TM * 128])   # [m*128+p, kc*128+j] = Cf[kc*128+p, m*128+j]
    din("Mi", [4 * 128, FM * 512])    # [n*128+p, kc*512+j] = Mi[kc*128+p, n*512+j]
    out_ap = nc.dram_tensor("out", [L, D], F32, kind="ExternalOutput").ap()
    q2 = nc.dram_tensor("q2", [D, 2 * L], BF16, kind="Internal").ap()

    with tile.TileContext(nc, trace_sim=False) as tc:
        _kernel_body(tc, dr, out_ap, q2)
    nc.compile()
    return nc


_NC_CACHE = {}


def _tile_w(W):
    return np.ascontiguousarray(
        np.asarray(W, np.float32).reshape(KC, 128, D).transpose(1, 0, 2).reshape(128, KC * D)
    )


def make_in_maps(q, k, Wq, bq, Wk, bk, Wo, bo):
    Cf, Mi = _build_dft_mats()
    # pre-tile so each stage does one big contiguous DMA per chunk column
    Cf = np.ascontiguousarray(
        Cf.reshape(TM, 128, FM, 128).transpose(2, 1, 0, 3).reshape(FM * 128, TM * 128)
    )
    Mi = np.ascontiguousarray(
        Mi.reshape(FM, 128, 4, 512).transpose(2, 1, 0, 3).reshape(4 * 128, FM * 512)
    )
    f32 = np.float32
    shared = {
        "Wq": _tile_w(Wq),
        "Wk": _tile_w(Wk),
        "Wo": _tile_w(Wo),
        "bq": np.ascontiguousarray(bq, f32).reshape(1, D),
        "bk": np.ascontiguousarray(bk, f32).reshape(1, D),
        "bo": np.ascontiguousarray(bo, f32).reshape(1, D),
        "bqc": np.ascontiguousarray(bq, f32).reshape(D, 1),
        "ones": np.ones((1, 128), f32),
        "ident": np.eye(128, dtype=np.float32).astype(__import__("ml_dtypes").bfloat16),
        "Cf": Cf,
        "Mi": Mi,
    }
    in_maps = []
    for b in range(B):
        m = dict(shared)
        m["qT"] = np.ascontiguousarray(np.asarray(q[b], f32).T)
        m["kT"] = np.ascontiguousarray(np.asarray(k[b], f32).T)
        in_maps.append(m)
    return in_maps


def kernel(q, k, v, Wq, bq, Wk, bk, Wv, bv, Wo, bo, _want_results=False,
           _trace=False, **_ignored):
    if "nc" not in _NC_CACHE:
        _NC_CACHE["nc"] = build_module()
    nc = _NC_CACHE["nc"]
    in_maps = make_in_maps(q, k, Wq, bq, Wk, bk, Wo, bo)
    res = run_bass_kernel_spmd(
        nc, in_maps, core_ids=list(range(N_CORES)), trace=_trace
    )
    out = np.stack([np.asarray(res.results[b]["out"], np.float32) for b in range(B)])
    if _want_results:
        return out, res
    return out


if __name__ == "__main__":
    # smoke test with random data
    rng = np.random.default_rng(0)
    q = rng.standard_normal((B, L, D), np.float32)
    k = rng.standard_normal((B, L, D), np.float32)
    s = 1.0 / np.sqrt(D)
    Wq = rng.standard_normal((D, D), np.float32) * s
    Wk = rng.standard_normal((D, D), np.float32) * s
    Wo = rng.standard_normal((D, D), np.float32) * s
    z = np.zeros(D, np.float32)
    out = kernel(q, k, None, Wq, z, Wk, z, None, None, Wo, z)
    print("out", out.shape, out.dtype, float(np.abs(out).sum()))


# revision 20
# speedup vs baseline: 1.2424x; 1.0831x over previous
"""AutoCorrelation block (FFT cross-correlation attention) on 8 Trainium2 cores.

Math (per batch b, faithfully reproducing the reference):
  qh = q @ Wq + bq, kh = k @ Wk + bk         (v projection is dead code)
  per channel c=(h,dh) (512 per batch):
    r = irfft(rfft(qh_c) * conj(rfft(kh_c)))   # circular cross-correlation
    top-8 lags d_k of r, softmax of the 8 values -> w_k
    agg_c[t] = sum_k w_k * qh_c[(t + d_k) % L]
  out = agg^T @ Wo + bo

Implementation: DFT-as-matmul with a stacked real cos/sin basis (the DFT matrix
is shared by all channels, so the whole FFT pipeline is dense PE work), DVE
max/max_index for top-8, and per-partition indirect-DMA gathers from a
time-doubled copy of qh for the mod-L rolls.

Sharding: data-parallel over batch. B == 8 == n_cores, one batch per core,
weights + DFT matrices replicated. No collectives.
"""

import numpy as np

import concourse.bass as bass
import concourse.bacc as bacc
import concourse.mybir as mybir
import concourse.tile as tile
from concourse.bass import IndirectOffsetOnAxis, ts
from concourse.bass_utils import run_bass_kernel_spmd

B, L, D = 8, 2048, 512
TOPK = 8
NF = 1025          # rfft bins for L=2048
FS = 2176          # stacked freq rows: 17 chunks of 128
IM0 = 1152         # sin(f) block starts here (f = 0..1023); rows 1025..1151 zero
N_CORES = 8
KC = 4             # d_in chunks of 128
TM = 16            # time chunks of 128
CN = 4             # channel chunks of 128
FM = 17            # stacked-freq chunks of 128

F32 = mybir.dt.float32
F32R = mybir.dt.float32r
U32 = mybir.dt.uint32
BF16 = mybir.dt.bfloat16
AF = mybir.ActivationFunctionType
AX = mybir.AxisListType


def _build_dft_mats():
    t = np.arange(L)
    f = np.arange(NF)
    ang = (2.0 * np.pi / L) * ((t[:, None] * f[None, :]) % L)
    Cf = np.zeros((L, FS), np.float32)
    Cf[:, :NF] = np.cos(ang)
    Cf[:, IM0 : IM0 + 1024] = np.sin(ang[:, :1024])
    ang2 = (2.0 * np.pi / L) * ((f[:, None] * t[None, :]) % L)
    Mi = np.zeros((FS, L), np.float32)
    Mi[0, :] = 1.0 / L
    Mi[1:1024, :] = (2.0 / L) * np.cos(ang2[1:1024])
    Mi[1024, :] = (1.0 / L) * np.cos(ang2[1024])
    Mi[IM0 : IM0 + 1024, :] = (2.0 / L) * np.sin(ang2[:1024])
    return Cf, Mi


def _kernel_body(tc, dr, out_ap, q2):
    nc = tc.nc

    w_pool = tc.alloc_tile_pool(name="weights", bufs=1)
    htd_pool = tc.alloc_tile_pool(name="htd", bufs=1, side="right")

    # ---- S1 inputs first so the PE can start ASAP ----
    qt_pool = tc.alloc_tile_pool(name="qt", bufs=1)
    qt = [qt_pool.tile([128, L], F32R, tag=f"qt{i}", name=f"qt{i}") for i in range(KC)]
    for i in range(KC):
        nc.sync.dma_start(qt[i][:, :], dr["qT"][ts(i, 128), :])

    # ---- constants ----
    wqk_pool = tc.alloc_tile_pool(name="wqk", bufs=1)
    wq_t = wqk_pool.tile([128, KC * D], F32R, tag="wqt", name="wqt")
    wk_t = wqk_pool.tile([128, KC * D], F32R, tag="wkt", name="wkt")
    wo_t = w_pool.tile([128, KC * D], F32R, tag="wot", name="wot")
    nc.sync.dma_start(wq_t[:, :], dr["Wq"][:, :])
    nc.sync.dma_start(wk_t[:, :], dr["Wk"][:, :])
    nc.sync.dma_start(wo_t[:, :], dr["Wo"][:, :])
    wq = [wq_t[:, ts(i, D)] for i in range(KC)]
    wk = [wk_t[:, ts(i, D)] for i in range(KC)]
    wo = [wo_t[:, ts(i, D)] for i in range(KC)]
    ones = w_pool.tile([1, 128], F32R, tag="ones", name="ones")
    nc.sync.dma_start(ones[:, :], dr["ones"][:, :])
    ident = w_pool.tile([128, 128], BF16, tag="ident", name="ident")
    nc.sync.dma_start(ident[:, :], dr["ident"][:, :])
    brow = {}
    for nm in ("bq", "bk", "bo", "bqL", "bkL"):
        brow[nm] = w_pool.tile([1, D], F32R, tag=f"{nm}r", name=f"{nm}r")
        nc.sync.dma_start(brow[nm][:, :], dr[nm][:, :])
    bqcol = w_pool.tile([128, CN], F32, tag="bqc", name="bqc")
    for c in range(CN):
        nc.sync.dma_start(bqcol[:, c : c + 1], dr["bqc"][ts(c, 128), :])

    htd_q = [htd_pool.tile([128, D], F32R, tag=f"hq{m}", name=f"hq{m}") for m in range(TM)]
    htd_k = [htd_pool.tile([128, D], F32R, tag=f"hk{m}", name=f"hk{m}") for m in range(TM)]

    # ---- S1/S2: projections ----
    ps1 = tc.alloc_tile_pool(name="ps1", bufs=4, space="PSUM")
    qht_pool = tc.alloc_tile_pool(name="qht", bufs=2)

    # qh_td[t, c] = sum_di qT[di, t] * Wq[di, c]  (+ bq broadcast row)
    for m in range(TM):
        ps = ps1.tile([128, D], F32, tag="p1", name="p1")
        for kc in range(KC):
            nc.tensor.matmul(
                ps[:, :], qt[kc][:, ts(m, 128)], wq[kc],
                start=(kc == 0), stop=(kc == KC - 1),
            )
        nc.scalar.activation(htd_q[m][:, :], ps[:, :], AF.Copy)
    # qh_t[c, t] (channel-major) -> q2 doubled in DRAM
    for mc in range(CN):
        qht = qht_pool.tile([128, L], BF16, tag="qht", name="qht")
        for n in range(4):
            ps = ps1.tile([128, 512], F32, tag="p1", name="p1")
            for kc in range(KC):
                nc.tensor.matmul(
                    ps[:, :], wq[kc][:, ts(mc, 128)], qt[kc][:, ts(n, 512)],
                    start=(kc == 0), stop=(kc == KC - 1),
                )
            nc.vector.tensor_scalar_add(
                qht[:, ts(n, 512)], ps[:, :], bqcol[:, mc : mc + 1]
            )
        nc.sync.dma_start(q2[ts(mc, 128), 0:L], qht[:, :])
        nc.sync.dma_start(q2[ts(mc, 128), L : 2 * L], qht[:, :])
    # kh_td
    kt = [qt_pool.tile([128, L], F32R, tag=f"qt{i}", name=f"kt{i}") for i in range(KC)]
    for i in range(KC):
        nc.sync.dma_start(kt[i][:, :], dr["kT"][ts(i, 128), :])
    for m in range(TM):
        ps = ps1.tile([128, D], F32, tag="p1", name="p1")
        for kc in range(KC):
            nc.tensor.matmul(
                ps[:, :], kt[kc][:, ts(m, 128)], wk[kc],
                start=(kc == 0), stop=(kc == KC - 1),
            )
        nc.scalar.activation(htd_k[m][:, :], ps[:, :], AF.Copy)

    qht_pool.release()
    ps1.release()
    wqk_pool.release()
    qt_pool.release()

    # ---- S3+S4 fused: forward DFT with inline freq product ----
    # Qhat[fs, c] = sum_t Cf[t, fs] * qh_td[t, c]; pairs (j, 9+j) are produced
    # back-to-back so Z = Qhat * conj(Khat) is computed inline and the big
    # Qhat/Khat buffers never materialize.
    s_pool0 = tc.alloc_tile_pool(name="small0", bufs=1)
    mi0_pool = tc.alloc_tile_pool(name="mi0", bufs=1)
    mi0_t = mi0_pool.tile([128, FM * 512], F32R, tag="mi0", name="mi0")
    nc.sync.dma_start(mi0_t[:, :], dr["Mi"][ts(0, 128), :])

    z_pool = tc.alloc_tile_pool(name="zfreq", bufs=1)
    cf_pool = tc.alloc_tile_pool(name="cf", bufs=2)
    f_pool = tc.alloc_tile_pool(name="fpair", bufs=6)
    ps3 = tc.alloc_tile_pool(name="ps3", bufs=2, space="PSUM")

    Z = [z_pool.tile([128, D], F32R, tag=f"z{j}", name=f"z{j}") for j in range(FM)]

    def dft_chunk(m):
        psq = ps3.tile([128, D], F32, tag="p3q", name="p3q")
        psk = ps3.tile([128, D], F32, tag="p3k", name="p3k")
        cf_t = cf_pool.tile([128, TM * 128], F32R, tag="cf", name="cf")
        nc.sync.dma_start(cf_t[:, :], dr["Cf"][ts(m, 128), :])
        for kc in range(TM):
            nc.tensor.matmul(
                psq[:, :], cf_t[:, ts(kc, 128)], htd_q[kc][:, :],
                start=(kc == 0), stop=(kc == TM - 1),
            )
            nc.tensor.matmul(
                psk[:, :], cf_t[:, ts(kc, 128)], htd_k[kc][:, :],
                start=(kc == 0), stop=(kc == TM - 1),
            )
        qf = f_pool.tile([128, D], F32R, tag="qf", name="qf")
        kf = f_pool.tile([128, D], F32R, tag="kf", name="kf")
        nc.scalar.activation(qf[:, :], psq[:, :], AF.Copy)
        nc.scalar.activation(kf[:, :], psk[:, :], AF.Copy)
        return qf, kf

    for j in range(8):
        re, im = j, 9 + j
        qf_a, kf_a = dft_chunk(re)
        if j == 0:
            nc.vector.tensor_add(qf_a[0:1, :], qf_a[0:1, :], brow["bqL"][:, :])
            nc.vector.tensor_add(kf_a[0:1, :], kf_a[0:1, :], brow["bkL"][:, :])
        qf_b, kf_b = dft_chunk(im)
        # Zre_j = Qre Kre + Qnim Knim ; Znim_j = Qnim Kre - Qre Knim
        t1 = f_pool.tile([128, D], F32R, tag="zt", name="zt")
        nc.vector.tensor_mul(Z[re][:, :], qf_a[:, :], kf_a[:, :])
        nc.gpsimd.tensor_mul(t1[:, :], qf_b[:, :], kf_b[:, :])
        nc.vector.tensor_add(Z[re][:, :], Z[re][:, :], t1[:, :])
        t2 = f_pool.tile([128, D], F32R, tag="zt", name="zt")
        nc.gpsimd.tensor_mul(t2[:, :], qf_b[:, :], kf_a[:, :])
        nc.vector.tensor_mul(Z[im][:, :], qf_a[:, :], kf_b[:, :])
        nc.vector.tensor_sub(Z[im][:, :], t2[:, :], Z[im][:, :])
    qf_n, kf_n = dft_chunk(8)
    nc.vector.tensor_mul(Z[8][:, :], qf_n[:, :], kf_n[:, :])

    ps3.release()
    f_pool.release()
    cf_pool.release()
    htd_pool.release()

    # ---- S5: inverse DFT  R[c, tau] = sum_fs Z[fs, c] * Mi[fs, tau] ----
    mi_pool = tc.alloc_tile_pool(name="mi", bufs=2)
    r_pool = tc.alloc_tile_pool(name="rcorr", bufs=1, side="right")
    ps5 = tc.alloc_tile_pool(name="ps5", bufs=2, space="PSUM")

    R = [r_pool.tile([128, L], F32, tag=f"r{m}", name=f"r{m}") for m in range(CN)]
    cand = [s_pool0.tile([128, 32], F32, tag=f"c{m}", name=f"c{m}") for m in range(CN)]
    for n in range(4):
        pss = [ps5.tile([128, 512], F32, tag=f"p5{m}", name=f"p5{m}") for m in range(CN)]
        if n == 0:
            mi_t = mi0_t
        else:
            mi_t = mi_pool.tile([128, FM * 512], F32R, tag="mi", name="mi")
            nc.sync.dma_start(mi_t[:, :], dr["Mi"][ts(n, 128), :])
        for kc in range(FM):
            for m in range(CN):
                nc.tensor.matmul(
                    pss[m][:, :], Z[kc][:, ts(m, 128)], mi_t[:, ts(kc, 512)],
                    start=(kc == 0), stop=(kc == FM - 1),
                )
        for m in range(CN):
            nc.scalar.activation(R[m][:, ts(n, 512)], pss[m][:, :], AF.Copy)
            # per-slice top-8 candidates, hidden under the next n's matmuls
            nc.vector.max(out=cand[m][:, ts(n, 8)], in_=R[m][:, ts(n, 512)])

    ps5.release()
    mi_pool.release()
    z_pool.release()
    mi0_pool.release()

    # ---- S6/S7: top-8 + softmax, then roll-gather + weighted sum.
    # The weighted sum runs on the PE: psum += I^T @ (w_k * G_k), keeping
    # the tensor engine warm through the tail instead of burning DVE adds.
    s_pool = tc.alloc_tile_pool(name="small", bufs=1)
    acc_pool = tc.alloc_tile_pool(name="acc", bufs=1, side="right")
    g_pool = tc.alloc_tile_pool(name="g", bufs=3)
    gm_pool = tc.alloc_tile_pool(name="gm", bufs=2)
    psa = tc.alloc_tile_pool(name="psa", bufs=8, space="PSUM")

    acc = [acc_pool.tile([128, L], F32R, tag=f"a{mc}", name=f"a{mc}") for mc in range(CN)]
    wsm = []
    offs = []
    for mc in range(CN):
        vals = s_pool.tile([128, 8], F32, tag=f"v{mc}", name=f"v{mc}")
        nc.vector.max(out=vals[:, :], in_=cand[mc][:, :])
        idx = s_pool.tile([128, 8], U32, tag=f"i{mc}", name=f"i{mc}")
        nc.vector.max_index(out=idx[:, :], in_max=vals[:, :], in_values=R[mc][:, :])
        negm = s_pool.tile([128, 1], F32, tag=f"nm{mc}", name=f"nm{mc}")
        nc.vector.tensor_scalar_mul(negm[:, :], vals[:, 0:1], -1.0)
        e = s_pool.tile([128, 8], F32, tag=f"e{mc}", name=f"e{mc}")
        nc.scalar.activation(e[:, :], vals[:, :], AF.Exp, bias=negm[:, :])
        ssum = s_pool.tile([128, 1], F32, tag=f"s{mc}", name=f"s{mc}")
        nc.vector.reduce_sum(out=ssum[:, :], in_=e[:, :], axis=AX.X)
        rs = s_pool.tile([128, 1], F32, tag=f"rs{mc}", name=f"rs{mc}")
        nc.vector.reciprocal(rs[:, :], ssum[:, :])
        wt = s_pool.tile([128, 8], F32, tag=f"w{mc}", name=f"w{mc}")
        nc.vector.tensor_scalar_mul(wt[:, :], e[:, :], rs[:, :])
        iob = s_pool.tile([128, 8], U32, tag=f"io{mc}", name=f"io{mc}")
        nc.gpsimd.iota(
            iob[:, :], pattern=[[0, 8]], base=mc * 128 * 2 * L,
            channel_multiplier=2 * L,
        )
        off = s_pool.tile([128, 8], U32, tag=f"o{mc}", name=f"o{mc}")
        nc.vector.tensor_add(off[:, :], idx[:, :], iob[:, :])
        wsm.append(wt)
        offs.append(off)

    for mc in range(CN):
        pacc = [psa.tile([128, 512], F32, tag="pa", name="pa") for _ in range(4)]
        for k in range(TOPK):
            g = g_pool.tile([128, L], BF16, tag="g", name="g")
            gi = nc.gpsimd.indirect_dma_start(
                out=g[:, :],
                out_offset=None,
                in_=q2[:, :],
                in_offset=IndirectOffsetOnAxis(ap=offs[mc][:, k : k + 1], axis=1),
            )
            if k % 4:
                gi.ins.queue = f"qPoolDynamic{k % 4}"
            gm = gm_pool.tile([128, L], BF16, tag="gm", name="gm")
            nc.scalar.activation(
                gm[:, :], g[:, :], AF.Identity, scale=wsm[mc][:, k : k + 1]
            )
            for nsl in range(4):
                nc.tensor.matmul(
                    pacc[nsl][:, :], ident[:, :], gm[:, ts(nsl, 512)],
                    start=(k == 0), stop=(k == TOPK - 1),
                )
        for nsl in range(4):
            nc.scalar.activation(acc[mc][:, ts(nsl, 512)], pacc[nsl][:, :], AF.Copy)

    psa.release()
    gm_pool.release()
    g_pool.release()
    s_pool.release()
    s_pool0.release()

    # ---- S8: output projection  out[t, :] = sum_c acc[c, t] * Wo[c, :] + bo ----
    po_pool = tc.alloc_tile_pool(name="po", bufs=1, space="PSUM")
    ot_pool = tc.alloc_tile_pool(name="ot", bufs=4)
    for half in range(2):
        pss = [po_pool.tile([128, D], F32, tag=f"po{m8}", name=f"po{m8}")
               for m8 in range(8)]
        for kc in range(CN):
            for m8 in range(8):
                nc.tensor.matmul(
                    pss[m8][:, :], acc[kc][:, ts(half * 8 + m8, 128)], wo[kc],
                    start=(kc == 0), stop=False,
                )
        for m8 in range(8):
            nc.tensor.matmul(
                pss[m8][:, :], ones[:, :], brow["bo"][:, :], start=False, stop=True
            )
            ot = ot_pool.tile([128, D], F32, tag="ot", name="ot")
            nc.scalar.activation(ot[:, :], pss[m8][:, :], AF.Copy)
            nc.sync.dma_start(out_ap[ts(half * 8 + m8, 128), :], ot[:, :])

    ot_pool.release()
    po_pool.release()
    acc_pool.release()
    r_pool.release()
    w_pool.release()


def build_module():
    nc = bacc.Bacc(
        "TRN2",
        target_bir_lowering=False,
        debug=False,
        enable_asserts=False,
        num_devices=N_CORES,
        num_swdge_queues=4,
    )
    dr = {}

    def din(name, shape, dt=F32R):
        dr[name] = nc.dram_tensor(name, shape, dt, kind="ExternalInput").ap()

    din("qT", [D, L])
    din("kT", [D, L])
    din("Wq", [128, KC * D])   # tiled: [p, kc*D+j] = W[kc*128+p, j]
    din("Wk", [128, KC * D])
    din("Wo", [128, KC * D])
    din("bq", [1, D])
    din("bk", [1, D])
    din("bo", [1, D])
    din("bqL", [1, D])
    din("bkL", [1, D])
    din("bqc", [D, 1], F32)
    din("ones", [1, 128])
    din("ident", [128, 128], BF16)
    din("Cf", [FM * 128, TM * 128])   # [m*128+p, kc*128+j] = Cf[kc*128+p, m*128+j]
    din("Mi", [4 * 128, FM * 512])    # [n*128+p, kc*512+j] = Mi[kc*128+p, n*512+j]
    out_ap = nc.dram_tensor("out", [L, D], F32, kind="ExternalOutput").ap()
    q2 = nc.dram_tensor("q2", [D, 2 * L], BF16, kind="Internal").ap()

    with tile.TileContext(nc, trace_sim=False) as tc:
        _kernel_body(tc, dr, out_ap, q2)
    nc.compile()
    return nc


_NC_CACHE = {}


def _tile_w(W):
    return np.ascontiguousarray(
        np.asarray(W, np.float32).reshape(KC, 128, D).transpose(1, 0, 2).reshape(128, KC * D)
    )


def make_in_maps(q, k, Wq, bq, Wk, bk, Wo, bo):
    Cf, Mi = _build_dft_mats()
    # pre-tile so each stage does one big contiguous DMA per chunk column
    Cf = np.ascontiguousarray(
        Cf.reshape(TM, 128, FM, 128).transpose(2, 1, 0, 3).reshape(FM * 128, TM * 128)
    )
    Mi = np.ascontiguousarray(
        Mi.reshape(FM, 128, 4, 512).transpose(2, 1, 0, 3).reshape(4 * 128, FM * 512)
    )
    f32 = np.float32
    shared = {
        "Wq": _tile_w(Wq),
        "Wk": _tile_w(Wk),
        "Wo": _tile_w(Wo),
        "bq": np.ascontiguousarray(bq, f32).reshape(1, D),
        "bk": np.ascontiguousarray(bk, f32).reshape(1, D),
        "bo": np.ascontiguousarray(bo, f32).reshape(1, D),
        "bqL": np.ascontiguousarray(np.asarray(bq, f32) * L, f32).reshape(1, D),
        "bkL": np.ascontiguousarray(np.asarray(bk, f32) * L, f32).reshape(1, D),
        "bqc": np.ascontiguousarray(bq, f32).reshape(D, 1),
        "ones": np.ones((1, 128), f32),
        "ident": np.eye(128, dtype=np.float32).astype(__import__("ml_dtypes").bfloat16),
        "Cf": Cf,
        "Mi": Mi,
    }
    in_maps = []
    for b in range(B):
        m = dict(shared)
        m["qT"] = np.ascontiguousarray(np.asarray(q[b], f32).T)
        m["kT"] = np.ascontiguousarray(np.asarray(k[b], f32).T)
        in_maps.append(m)
    return in_maps


def kernel(q, k, v, Wq, bq, Wk, bk, Wv, bv, Wo, bo, _want_results=False,
           _trace=False, **_ignored):
    if "nc" not in _NC_CACHE:
        _NC_CACHE["nc"] = build_module()
    nc = _NC_CACHE["nc"]
    in_maps = make_in_maps(q, k, Wq, bq, Wk, bk, Wo, bo)
    res = run_bass_kernel_spmd(
        nc, in_maps, core_ids=list(range(N_CORES)), trace=_trace
    )
    out = np.stack([np.asarray(res.results[b]["out"], np.float32) for b in range(B)])
    if _want_results:
        return out, res
    return out


if __name__ == "__main__":
    # smoke test with random data
    rng = np.random.default_rng(0)
    q = rng.standard_normal((B, L, D), np.float32)
    k = rng.standard_normal((B, L, D), np.float32)
    s = 1.0 / np.sqrt(D)
    Wq = rng.standard_normal((D, D), np.float32) * s
    Wk = rng.standard_normal((D, D), np.float32) * s
    Wo = rng.standard_normal((D, D), np.float32) * s
    z = np.zeros(D, np.float32)
    out = kernel(q, k, None, Wq, z, Wk, z, None, None, Wo, z)
    print("out", out.shape, out.dtype, float(np.abs(out).sum()))
